# revision 27
# baseline (speedup 1.0000x reference)
"""AttentionWithContext pooling kernel for Trainium2 (8 NeuronCores, data parallel).

Computes, for x[B,T,F], W[F,F], b[F], u[F]:
    uit = tanh(x @ W + b)           [B,T,F]
    ait = uit . u                   [B,T]
    a   = exp(ait); a /= (sum_T a + 1e-7)
    out = sum_T a * x               [B,F]

Sharding: pure data parallel over batch B=128 -> 16 batches per core.

Per-core dataflow (all matmul inputs bf16, fp32 accumulation):
  - SWDGE DMA loads x with fp32->bf16 cast, natural layout [128 rows, 256 f]
  - XBAR DMA transpose produces xT tiles [128 f, rows] for the main matmul
  - PE: z^T[g, r] = sum_f W[f,g] xT[f,r]  (W stationary)
  - ACT: uitT = tanh(z^T + b)  (bias fused, per-partition)
  - PE: ait column form: lhsT=uitT slice [128 g, 128 r], rhs=u -> psum [128 r, 1]
  - ACT: a = exp(ait) batched [128, 16] with accum_out -> per-partition denom sums
  - PE: numerator[f] = sum_r a_r x[r,f]  (a stationary column, x natural as rhs)
  - PE: denominators: ones-matmul collapses denom partials [128,16] -> [16,1]
  - DVE: reciprocal(denom+eps), scale numerator rows, DMA out [16, 256]
"""

import sys

for _p in ("/opt/trn_rl_repo",):
    if _p not in sys.path:
        sys.path.insert(0, _p)

from contextlib import ExitStack

import numpy as np

import concourse.bass as bass
import concourse.mybir as mybir
import concourse.tile as tile
from concourse import bacc
from concourse.bass_utils import run_bass_kernel_spmd

B, T, F = 128, 2048, 256
NCORES = 8
BS = B // NCORES  # batches per core
P = 128
SUBT = T // P  # 16 row-subtiles of 128 per batch
NGRP = 2  # groups of 8 subtiles (1024 rows) per batch
EPS = 1e-7

FP32 = mybir.dt.float32
BF16 = mybir.dt.bfloat16
AF = mybir.ActivationFunctionType

DEBUG_TAPS = False
REPS = 1  # bench knob: repeat the whole per-core computation REPS times


def _kernel_body(tc, x, W, bvec, u, out, taps=None):
    nc = tc.nc
    ctx = ExitStack()
    with ctx:
        singles = ctx.enter_context(tc.tile_pool(name="singles", bufs=1))
        dram = ctx.enter_context(tc.tile_pool(name="dram", bufs=1, space="DRAM"))
        xnat_pool = ctx.enter_context(tc.tile_pool(name="xnat", bufs=5))
        xt_pool = ctx.enter_context(tc.tile_pool(name="xt", bufs=4))
        uit_pool = ctx.enter_context(tc.tile_pool(name="uit", bufs=3))
        small_pool = ctx.enter_context(tc.tile_pool(name="small", bufs=3))
        psum_z = ctx.enter_context(tc.tile_pool(name="psz", bufs=2, space="PSUM"))
        psum_ait = ctx.enter_context(tc.tile_pool(name="psait", bufs=2, space="PSUM"))
        psum_num = ctx.enter_context(tc.tile_pool(name="psnum", bufs=1, space="PSUM"))
        psum_den = ctx.enter_context(tc.tile_pool(name="psden", bufs=1, space="PSUM"))

        # --- constants ---
        # W[f, g] split into f-chunks c: W_sb[p, c, g] = W[c*128+p, g]
        W_sb = singles.tile([P, 2, F], BF16)
        nc.gpsimd.dma_start(W_sb, W.rearrange("(c p) g -> p c g", p=P))
        # u split into g-chunks h: u_sb[p, h] = u[h*128+p]
        u_sb = singles.tile([P, 2], BF16)
        nc.gpsimd.dma_start(u_sb, u.rearrange("(h p) -> p h", p=P))
        # bias, fp32 per-partition for the tanh
        b_sb = singles.tile([P, 2], FP32)
        nc.sync.dma_start(b_sb, bvec.rearrange("(h p) -> p h", p=P))
        ones_f = singles.tile([P, 1], FP32)
        nc.vector.memset(ones_f, 1.0)

        if taps is not None:
            dbg_ait = singles.tile([P, BS * SUBT], FP32)
            dbg_a = singles.tile([P, BS * SUBT], FP32)
            dbg_uit = singles.tile([P, 2, 1024], BF16)
            dbg_xt = singles.tile([P, 8, 2, P], BF16)

        # per-batch softmax denominator partials (per partition), batch on free dim
        denoms = singles.tile([P, BS], FP32)
        # numerator staging, flat on partition 0 (compute-engine writes must
        # start at a 32-aligned partition, so batch can't go on partitions yet)
        num_flat = singles.tile([1, BS * F], FP32)

        def emit_numerator(bb, a_sb, xnat):
            # numerator: psn[0, f] = sum_st sum_p a[p, st] * x[st*128+p, f]
            psn = psum_num.tile([1, F], FP32, tag="psn")
            for st in range(SUBT):
                nc.tensor.matmul(
                    psn,
                    lhsT=a_sb[:, st : st + 1],
                    rhs=xnat[:, st, :],
                    start=(st == 0),
                    stop=(st == SUBT - 1),
                )
            nc.vector.tensor_copy(num_flat[:, bb * F : (bb + 1) * F], psn)

        def emit_ait(unit):
            # ait column form per 128-row slice; runs one unit after its
            # tanh so PE never stalls on the ACT pipeline
            bb, g, uitT0, uitT1, ait_st = unit
            pait = psum_ait.tile([P, 8], FP32, tag="pait")
            for s in range(8):
                for h, ut in ((0, uitT0), (1, uitT1)):
                    nc.tensor.matmul(
                        pait[:, s : s + 1],
                        lhsT=ut[:, s * P : (s + 1) * P],
                        rhs=u_sb[:, h : h + 1],
                        start=(h == 0),
                        stop=(h == 1),
                    )
            nc.vector.tensor_copy(ait_st[:, g * 8 : (g + 1) * 8], pait)

        def emit_exp(bb, ait_st):
            # exp over the whole batch; accum_out gives per-partition
            # denominator partials for free
            a_sb = small_pool.tile([P, SUBT], BF16, tag="a_sb")
            nc.scalar.activation(
                a_sb, ait_st, AF.Exp, accum_out=denoms[:, bb : bb + 1]
            )
            if taps is not None:
                nc.vector.tensor_copy(
                    dbg_ait[:, bb * SUBT : (bb + 1) * SUBT], ait_st
                )
                nc.vector.tensor_copy(
                    dbg_a[:, bb * SUBT : (bb + 1) * SUBT], a_sb
                )
            return a_sb

        pending_ait = None  # unit whose ait matmuls haven't been emitted
        pending_num = None  # (bb, a_sb, xnat) awaiting numerator
        batch_state = {}  # bb -> (xnat, ait_st)

        for bb_rep in range(BS * REPS):
            bb = bb_rep % BS
            # load full batch, cast fp32->bf16. xnat[p, s, f] = x[bb, s*128+p, f]
            xnat = xnat_pool.tile([P, SUBT, F], BF16)
            nc.gpsimd.dma_start(xnat, x[bb].rearrange("(s p) f -> p s f", p=P))
            if taps is not None and bb == 0:
                nc.sync.dma_start(taps["dbg_xnat0"], xnat)

            ait_st = small_pool.tile([P, SUBT], FP32, tag="ait_st")
            batch_state[bb] = (xnat, ait_st)

            # one batched xbar transpose for the whole batch:
            # xt[p_f, s, c, r] = xnat[r, s, c*128+p_f]
            xt = xt_pool.tile([P, SUBT, 2, P], BF16)
            nc.sync.dma_start_transpose(
                xt.rearrange("p s c r -> p (s c) r"),
                xnat.rearrange("p s f -> p (s f)"),
            )
            if taps is not None and bb == 0:
                nc.vector.tensor_copy(dbg_xt, xt[:, :8])

            for g in range(NGRP):
                for h in range(2):
                    zps = psum_z.tile([P, 1024], FP32, tag="zps")
                    for c in range(2):
                        for nhalf in range(2):
                            nc.tensor.matmul(
                                zps[:, nhalf * 512 : (nhalf + 1) * 512],
                                lhsT=W_sb[:, c, h * P : (h + 1) * P],
                                rhs=xt[:, g * 8 + 4 * nhalf : g * 8 + 4 * nhalf + 4, c, :],
                                start=(c == 0),
                                stop=(c == 1),
                            )
                    uitT = uit_pool.tile([P, 1024], BF16, tag=f"uitT{h}")
                    nc.scalar.activation(
                        uitT, zps, AF.Tanh, bias=b_sb[:, h : h + 1], scale=1.0
                    )
                    if taps is not None and bb == 0 and g == 0:
                        nc.vector.tensor_copy(dbg_uit[:, h], uitT)
                    if h == 0:
                        uitT0 = uitT
                    else:
                        uitT1 = uitT

                # previous batch's numerator: its exp has had a full unit of
                # PE work to complete on ACT
                if pending_num is not None:
                    emit_numerator(*pending_num)
                    pending_num = None

                # previous unit's ait: its tanh ran during this unit's matmuls
                if pending_ait is not None:
                    pbb, pg = pending_ait[0], pending_ait[1]
                    emit_ait(pending_ait)
                    if pg == NGRP - 1:
                        pxnat, pait_st = batch_state.pop(pbb)
                        a_sb = emit_exp(pbb, pait_st)
                        pending_num = (pbb, a_sb, pxnat)
                pending_ait = (bb, g, uitT0, uitT1, ait_st)

        # drain the pipeline tail
        emit_ait(pending_ait)
        lxnat, lait_st = batch_state.pop(BS - 1)
        if pending_num is not None:
            emit_numerator(*pending_num)
        a_sb = emit_exp(BS - 1, lait_st)
        emit_numerator(BS - 1, a_sb, lxnat)

        # scatter numerators to batch-on-partitions layout via a DRAM bounce
        # (SBUF->SBUF partition scatter via AP rearrange is not HW-realizable)
        num_dram = dram.tile([BS, F], FP32)
        nc.sync.dma_start(num_dram, num_flat)
        num_t = singles.tile([BS, F], FP32)
        nc.sync.dma_start(num_t, num_dram)
        # collapse denominator partials: psd[bb, 0] = sum_p denoms[p, bb]
        psd = psum_den.tile([BS, 1], FP32, tag="psd")
        nc.tensor.matmul(psd, lhsT=denoms, rhs=ones_f, start=True, stop=True)
        den_sb = singles.tile([BS, 1], FP32)
        nc.vector.tensor_copy(den_sb, psd)
        nc.vector.tensor_scalar_add(den_sb, den_sb, EPS)
        nc.vector.reciprocal(den_sb, den_sb)
        out_sb = singles.tile([BS, F], FP32)
        nc.vector.tensor_scalar_mul(out_sb, num_t, den_sb)
        nc.sync.dma_start(out, out_sb)

        if taps is not None:
            nc.sync.dma_start(taps["dbg_ait"], dbg_ait)
            nc.sync.dma_start(taps["dbg_a"], dbg_a)
            nc.sync.dma_start(taps["dbg_uit"], dbg_uit)
            nc.sync.dma_start(taps["dbg_xt"], dbg_xt)
            nc.sync.dma_start(taps["dbg_denoms"], denoms)
            nc.sync.dma_start(taps["dbg_num"], num_flat)
            nc.sync.dma_start(taps["dbg_numt"], num_t)
            nc.sync.dma_start(taps["dbg_den"], den_sb)


_CACHED_NC = None


def _build_nc():
    global _CACHED_NC
    if _CACHED_NC is not None:
        return _CACHED_NC
    nc = bacc.Bacc(
        "TRN2",
        target_bir_lowering=False,
        debug=False,
        enable_asserts=False,
        num_devices=NCORES,
    )
    x_d = nc.dram_tensor("x", [BS, T, F], FP32, kind="ExternalInput")
    W_d = nc.dram_tensor("W", [F, F], FP32, kind="ExternalInput")
    b_d = nc.dram_tensor("b", [F], FP32, kind="ExternalInput")
    u_d = nc.dram_tensor("u", [F], FP32, kind="ExternalInput")
    o_d = nc.dram_tensor("out", [BS, F], FP32, kind="ExternalOutput")
    taps = None
    if DEBUG_TAPS:
        taps = {
            "dbg_ait": nc.dram_tensor(
                "dbg_ait", [P, BS * SUBT], FP32, kind="ExternalOutput"
            ).ap(),
            "dbg_a": nc.dram_tensor(
                "dbg_a", [P, BS * SUBT], FP32, kind="ExternalOutput"
            ).ap(),
            "dbg_uit": nc.dram_tensor(
                "dbg_uit", [P, 2, 1024], BF16, kind="ExternalOutput"
            ).ap(),
            "dbg_xt": nc.dram_tensor(
                "dbg_xt", [P, 8, 2, P], BF16, kind="ExternalOutput"
            ).ap(),
            "dbg_denoms": nc.dram_tensor(
                "dbg_denoms", [P, BS], FP32, kind="ExternalOutput"
            ).ap(),
            "dbg_num": nc.dram_tensor(
                "dbg_num", [1, BS * F], FP32, kind="ExternalOutput"
            ).ap(),
            "dbg_xnat0": nc.dram_tensor(
                "dbg_xnat0", [P, SUBT, F], BF16, kind="ExternalOutput"
            ).ap(),
            "dbg_numt": nc.dram_tensor(
                "dbg_numt", [BS, F], FP32, kind="ExternalOutput"
            ).ap(),
            "dbg_den": nc.dram_tensor(
                "dbg_den", [BS, 1], FP32, kind="ExternalOutput"
            ).ap(),
        }
    with tile.TileContext(nc) as tc:
        _kernel_body(tc, x_d.ap(), W_d.ap(), b_d.ap(), u_d.ap(), o_d.ap(), taps)
    nc.compile()
    _CACHED_NC = nc
    return nc


_CACHED_RUNNER = None


def _build_runner():
    """Build a cached sharded-jit callable over 8 cores.

    Mirrors bass2jax.run_bass_via_pjrt's multi-core branch, but keeps the
    jitted function (and input layout logic) so repeated calls don't re-trace.
    """
    global _CACHED_RUNNER
    if _CACHED_RUNNER is not None:
        return _CACHED_RUNNER

    import jax
    from jax.experimental.shard_map import shard_map
    from jax.sharding import Mesh, PartitionSpec

    from concourse import bass2jax as b2j
    from concourse import mybir as _mybir

    nc = _build_nc()
    b2j.install_neuronx_cc_hook()
    assert nc.dbg_addr is None

    partition_name = (
        nc.partition_id_tensor.name if nc.partition_id_tensor else None
    )
    in_names = []
    out_names = []
    out_shapes = []
    for alloc in nc.m.functions[0].allocations:
        if not isinstance(alloc, _mybir.MemoryLocationSet):
            continue
        name = alloc.memorylocations[0].name
        if alloc.kind == "ExternalInput":
            if name != partition_name:
                in_names.append(name)
        elif alloc.kind == "ExternalOutput":
            out_names.append(name)
            out_shapes.append(
                (tuple(alloc.tensor_shape), _mybir.dt.np(alloc.dtype))
            )
    n_params = len(in_names)
    n_outs = len(out_names)
    out_avals = [
        jax.core.ShapedArray(shape, dtype) for shape, dtype in out_shapes
    ]
    all_names = tuple(in_names + out_names)
    if partition_name is not None:
        all_names = all_names + (partition_name,)

    def _body(*args):
        operands = list(args)
        if partition_name is not None:
            operands.append(b2j.partition_id_tensor())
        outs = b2j._bass_exec_p.bind(
            *operands,
            out_avals=tuple(out_avals),
            in_names=all_names,
            out_names=tuple(out_names),
            lowering_input_output_aliases=(),
            sim_require_finite=True,
            sim_require_nnan=True,
            nc=nc,
        )
        return tuple(outs)

    devices = jax.devices()[:NCORES]
    mesh = Mesh(np.asarray(devices), ("core",))
    in_specs = (PartitionSpec("core"),) * (n_params + n_outs)
    out_specs = (PartitionSpec("core"),) * n_outs
    donate = tuple(range(n_params, n_params + n_outs))
    sharded = jax.jit(
        shard_map(
            _body, mesh=mesh, in_specs=in_specs, out_specs=out_specs,
            check_rep=False,
        ),
        donate_argnums=donate,
        keep_unused=True,
    )

    def run(per_core_inputs):
        # per_core_inputs: dict name -> list of 8 per-core np arrays
        concat_in = [
            np.concatenate(per_core_inputs[name], axis=0) for name in in_names
        ]
        zeros = [
            np.zeros((NCORES * s[0], *s[1:]), dt) for (s, dt) in out_shapes
        ]
        out_arrs = sharded(*concat_in, *zeros)
        res = {}
        for i, name in enumerate(out_names):
            shape, _ = out_shapes[i]
            res[name] = np.asarray(out_arrs[i]).reshape(NCORES, *shape)
        return res

    _CACHED_RUNNER = (run, sharded, in_names, out_names, out_shapes)
    return _CACHED_RUNNER


def _split_inputs(inputs):
    x = np.ascontiguousarray(np.asarray(inputs["x"], dtype=np.float32))
    W = np.ascontiguousarray(np.asarray(inputs["W"], dtype=np.float32))
    b = np.ascontiguousarray(np.asarray(inputs["b"], dtype=np.float32))
    u = np.ascontiguousarray(np.asarray(inputs["u"], dtype=np.float32))
    return {
        "x": [x[c * BS : (c + 1) * BS] for c in range(NCORES)],
        "W": [W] * NCORES,
        "b": [b] * NCORES,
        "u": [u] * NCORES,
    }


def kernel(**inputs):
    run, *_ = _build_runner()
    res = run(_split_inputs(inputs))
    out = res["out"].reshape(B, F)
    return out.astype(np.float32)
